# revision 1
# baseline (speedup 1.0000x reference)
"""DOMINO++ loss kernel for Trainium2 (8 NeuronCores, data-parallel).

Strategy
--------
Shard the (n=2, c=12, 96^3) logits over 8 cores: 4 z-major contiguous
spatial blocks per batch element (cores 0-3 -> n=0, cores 4-7 -> n=1).
Each core reduces its 221184 voxels to a small set of partial sums:

  - psum matrix  out[13, 24] accumulated on the TensorEngine:
      lhsT (stationary) = [onehot masks m_t (12, bf16), ones] -> 13 cols
      rhs  (moving)     = [probs g_c (12, bf16), logits x_c (12, bf16)]
      out[t, c]    = sum_v m_t(v) g_c(v)      (= M, feeds dice inter/ground
                                               and the penalty Frobenius sum)
      out[t, 12+t] = sum_{v in t} x_t(v)      (CE logit-gather term)
  - predacc[128, NCH*12] f32: per-class sums of probs (DVE accum_out, exact)
  - logdacc[128, NCH]    f32: per-voxel log(sum exp) row sums (ACT accum_out)

Host combines the 8 cores' outputs (a few KB) into the scalar loss.

Per-voxel pipeline on device (all tiles [128, F] voxel-major):
  ACT: exp x12 (in-place), ln(denom) + accum, recip = exp(-ln d)
  DVE: denom = strided reduce over class axis; g_c = exp_c * recip (bf16 out,
       f32 accum -> pred)
  POOL: masks (t == tgt) -> bf16; part of the bf16 logit copies
  PE:  1728 accumulating [128,13]x[128,24] bf16 matmuls
"""

import os
import sys
from contextlib import ExitStack

import numpy as np

sys.path.insert(0, "/opt/trn_rl_repo")

from concourse import bacc, bass, mybir, tile  # noqa: E402
from concourse import bass_utils  # noqa: E402

F32 = mybir.dt.float32
BF16 = mybir.dt.bfloat16
ALU = mybir.AluOpType
ACTF = mybir.ActivationFunctionType

N_CORES = 8
C = 12            # classes
P = 128           # SBUF partitions
FT = 1728         # free size per partition per core (P*FT = 221184 voxels)
NCH = 4           # chunks
FC = FT // NCH    # free columns per chunk
S = P * FT        # voxels per core
N, H, W, Z = 2, 96, 96, 96
SPATIAL = H * W * Z          # 884736 voxels per batch element
CORES_PER_N = N_CORES // N   # 4

# engine assignment for the 12 bf16 logit copies (tunable: 'act'/'dve'/'pool')
XCOPY_ENGINES = ["act", "act", "act", "act",
                 "dve", "dve", "dve", "dve",
                 "pool", "pool", "pool", "pool"]

_CACHE = {}


def _build_program():
    """Build + compile the per-core Bass program (identical on all cores)."""
    nc = bacc.Bacc("TRN2", target_bir_lowering=False, debug=False,
                   num_devices=N_CORES)

    x_d = nc.dram_tensor("x", (C, S), F32, kind="ExternalInput")
    t_d = nc.dram_tensor("t", (S,), F32, kind="ExternalInput")
    m_d = nc.dram_tensor("m_out", (13, 24), F32, kind="ExternalOutput")
    pred_d = nc.dram_tensor("pred_out", (P, NCH * C), F32, kind="ExternalOutput")
    logd_d = nc.dram_tensor("logd_out", (P, NCH), F32, kind="ExternalOutput")

    x_v = x_d.rearrange("c (p f) -> c p f", p=P)
    t_v = t_d.rearrange("(p f) -> p f", p=P)

    with ExitStack() as ctx:
        tc = ctx.enter_context(tile.TileContext(nc))
        sb = ctx.enter_context(tc.tile_pool(name="sb", bufs=2))
        acc = ctx.enter_context(tc.tile_pool(name="acc", bufs=1))
        ps = ctx.enter_context(tc.tile_pool(name="ps", bufs=1, space="PSUM"))

        predacc = acc.tile([P, NCH * C], F32)
        logdacc = acc.tile([P, NCH], F32)
        psum_m = ps.tile([13, 24], F32)

        for ch in range(NCH):
            lo = ch * FC
            xt = sb.tile([P, C, FC], F32, tag="xt")
            tt = sb.tile([P, FC], F32, tag="tt")
            gx = sb.tile([P, 24, FC], BF16, tag="gx")
            mk = sb.tile([P, 13, FC], BF16, tag="mk")
            dn = sb.tile([P, FC], F32, tag="dn")
            lg = sb.tile([P, FC], F32, tag="lg")
            rc = sb.tile([P, FC], F32, tag="rc")

            # ---- loads ----
            for c in range(C):
                nc.sync.dma_start(xt[:, c], x_v[c, :, lo:lo + FC])
            nc.sync.dma_start(tt[:], t_v[:, lo:lo + FC])

            # ---- bf16 copies of raw logits (before in-place exp) ----
            for c in range(C):
                eng = XCOPY_ENGINES[c]
                if eng == "act":
                    nc.scalar.copy(gx[:, 12 + c], xt[:, c])
                elif eng == "dve":
                    nc.vector.tensor_copy(gx[:, 12 + c], xt[:, c])
                else:
                    nc.gpsimd.tensor_copy(gx[:, 12 + c], xt[:, c])

            # ---- masks m_t = (tgt == t) and the ones plane ----
            for t in range(C):
                nc.gpsimd.tensor_scalar(mk[:, t], tt[:], float(t), None,
                                        op0=ALU.is_equal)
            nc.vector.memset(mk[:, 12], 1.0)

            # ---- softmax pieces ----
            for c in range(C):
                nc.scalar.activation(xt[:, c], xt[:, c], ACTF.Exp)
            # denom over the class axis (strided innermost reduce)
            nc.vector.tensor_reduce(dn[:], xt[:].rearrange("p c f -> p f c"),
                                    axis=mybir.AxisListType.X, op=ALU.add)
            nc.scalar.activation(lg[:], dn[:], ACTF.Ln,
                                 accum_out=logdacc[:, ch:ch + 1])
            nc.scalar.activation(rc[:], lg[:], ACTF.Exp, scale=-1.0)

            # ---- probs g_c = exp_c * recip (bf16 out, exact f32 row sums) ----
            for c in range(C):
                nc.vector.scalar_tensor_tensor(
                    gx[:, c], xt[:, c], 1.0, rc[:],
                    op0=ALU.mult, op1=ALU.mult,
                    accum_out=predacc[:, ch * C + c:ch * C + c + 1])

            # ---- accumulate the 13x24 statistics matrix on PE ----
            for j in range(FC):
                nc.tensor.matmul(psum_m[:], mk[:, :, j], gx[:, :, j],
                                 start=(ch == 0 and j == 0),
                                 stop=(ch == NCH - 1 and j == FC - 1))

        m_sb = acc.tile([13, 24], F32)
        nc.vector.tensor_copy(m_sb[:], psum_m[:])
        nc.sync.dma_start(m_d[:], m_sb[:])
        nc.sync.dma_start(pred_d[:], predacc[:])
        nc.sync.dma_start(logd_d[:], logdacc[:])

    nc.compile()
    return nc


def _get_program():
    if "nc" not in _CACHE:
        _CACHE["nc"] = _build_program()
    return _CACHE["nc"]


def _shard_inputs(input, target):
    """Full (2,12,96,96,96)/(2,1,96,96,96) -> 8 per-core in_maps."""
    x = np.ascontiguousarray(np.asarray(input, dtype=np.float32))
    tg = np.asarray(target).reshape(N, SPATIAL)
    in_maps = []
    for k in range(N_CORES):
        n = k // CORES_PER_N
        o = (k % CORES_PER_N) * S
        xs = np.ascontiguousarray(x[n].reshape(C, SPATIAL)[:, o:o + S])
        ts = np.ascontiguousarray(tg[n, o:o + S]).astype(np.float32)
        in_maps.append({"x": xs, "t": ts})
    return in_maps


def _combine(results, matrix_penalty, global_step, maxiter):
    pen = np.asarray(matrix_penalty, dtype=np.float64)
    inter = np.zeros((N, C))
    ground = np.zeros((N, C))
    pred = np.zeros((N, C))
    xtgt_sum = 0.0
    logd_sum = 0.0
    pen_sum = 0.0
    for k, r in enumerate(results):
        n = k // CORES_PER_N
        m = np.asarray(r["m_out"], dtype=np.float64)
        mg = m[:C, :C]                       # sum_v m_t * g_c
        inter[n] += np.diag(mg)
        ground[n] += mg.sum(axis=1)
        pred[n] += np.asarray(r["pred_out"], dtype=np.float64) \
            .reshape(P, NCH, C).sum(axis=(0, 1))
        xtgt_sum += np.trace(m[:C, C:2 * C])
        logd_sum += float(np.asarray(r["logd_out"], dtype=np.float64).sum())
        pen_sum += float((pen * mg).sum())

    nvox = N * SPATIAL
    dice = 1.0 - (2.0 * inter + 1e-5) / (ground + pred + 1e-5)
    dice_loss = dice.mean()
    ce = (logd_sum - xtgt_sum) / nvox
    ce_total = dice_loss + ce
    pen_mean = pen_sum / nvox
    beta = 10.0 ** np.floor(np.log10(ce_total))
    gs = float(global_step)
    mi = float(maxiter)
    alpha0 = 1.0 - gs / mi
    alpha1 = gs / mi
    return np.float32(alpha1 * ce_total + alpha0 * beta * pen_mean)


def kernel(input, target, matrix_penalty, global_step, maxiter):
    nc = _get_program()
    in_maps = _shard_inputs(input, target)
    trace = bool(int(os.environ.get("BASS_LOSS_TRACE", "0")))
    res = bass_utils.run_bass_kernel_spmd(
        nc, in_maps, core_ids=list(range(N_CORES)), trace=trace)
    _CACHE["last_exec_ns"] = res.exec_time_ns
    return _combine(res.results, matrix_penalty, global_step, maxiter)


# revision 6
# speedup vs baseline: 3.3381x; 3.3381x over previous
"""DOMINO++ loss kernel for Trainium2 (8 NeuronCores, data-parallel).

Strategy
--------
Shard the (n=2, c=12, 96^3) logits over 8 cores: 4 contiguous spatial
blocks per batch element (cores 0-3 -> n=0, cores 4-7 -> n=1).
Each core reduces its 221184 voxels to a small set of partial sums:

  - A [13, 24] statistics matrix accumulated on the TensorEngine from
    bf16 operands (9 voxel-columns batched per matmul; the [117, 216]
    PSUM holds 9 diagonal [13, 24] blocks that the host sums):
      lhsT (stationary) = [onehot masks m_t (12), ones]
      rhs  (moving)     = [probs g_c (12), logits x_c (12)]
      out[t, c]    = sum_v m_t(v) g_c(v)   (dice inter = diag, ground =
                                            row sums, penalty = Frobenius
                                            product with the penalty matrix)
      out[t, 12+t] = sum_{v in t} x_t(v)   (CE logit-gather term)
      out[12, c]   = sum_v g_c(v)          (dice pred)
  - logdacc[128, NCH] f32: row sums of log(sum_c exp x_c) (ACT accum_out)

Host combines the 8 cores' outputs (a few KB) into the scalar loss.

Per-voxel pipeline on device (tiles [128, F] voxel-major):
  ACT: exp x12 (in-place), ln(denom) + accum, recip = exp(-ln d),
       half the bf16 logit copies
  DVE: masks (t == tgt) -> bf16; denom (strided class reduce);
       g_c = exp_c * recip -> bf16; other half of the bf16 copies
  PE:  192 accumulating [128,117] x [128,216] bf16 matmuls
"""

import os
import sys
from contextlib import ExitStack

import numpy as np

sys.path.insert(0, "/opt/trn_rl_repo")

from concourse import bacc, bass, mybir, tile  # noqa: E402
from concourse import bass_utils  # noqa: E402

F32 = mybir.dt.float32
BF16 = mybir.dt.bfloat16
ALU = mybir.AluOpType
ACTF = mybir.ActivationFunctionType

N_CORES = 8
C = 12            # classes
P = 128           # SBUF partitions
FT = 1728         # free size per partition per core (P*FT = 221184 voxels)
NCH = 4           # chunks
FC = FT // NCH    # free columns per chunk
JB = 9            # voxel-columns batched per matmul (13*JB <= 128)
S = P * FT        # voxels per core
N, H, W, Z = 2, 96, 96, 96
SPATIAL = H * W * Z          # 884736 voxels per batch element
CORES_PER_N = N_CORES // N   # 4

_CACHE = {}


def _build_program():
    """Build + compile the per-core Bass program (identical on all cores)."""
    nc = bacc.Bacc("TRN2", target_bir_lowering=False, debug=False,
                   num_devices=N_CORES)

    x_d = nc.dram_tensor("x", (C, S), F32, kind="ExternalInput")
    t_d = nc.dram_tensor("t", (S,), F32, kind="ExternalInput")
    m_d = nc.dram_tensor("m_out", (13 * JB, 24 * JB), F32,
                         kind="ExternalOutput")
    logd_d = nc.dram_tensor("logd_out", (P, NCH), F32, kind="ExternalOutput")

    x_v = x_d.rearrange("c (p f) -> c p f", p=P)
    t_v = t_d.rearrange("(p f) -> p f", p=P)

    with ExitStack() as ctx:
        tc = ctx.enter_context(tile.TileContext(nc))
        sb = ctx.enter_context(tc.tile_pool(name="sb", bufs=2))
        acc = ctx.enter_context(tc.tile_pool(name="acc", bufs=1))
        ps = ctx.enter_context(tc.tile_pool(name="ps", bufs=1, space="PSUM"))

        logdacc = acc.tile([P, NCH], F32)
        psum_m = ps.tile([13 * JB, 24 * JB], F32)

        for ch in range(NCH):
            lo = ch * FC
            xt = sb.tile([P, C, FC], F32, tag="xt")
            tt = sb.tile([P, FC], F32, tag="tt")
            # (group, plane, j) layout so each matmul's 117/216 stationary/
            # moving columns are contiguous (walrus: single free dim only)
            gx = sb.tile([P, FC // JB, 24, JB], BF16, tag="gx")
            mk = sb.tile([P, FC // JB, 13, JB], BF16, tag="mk")
            dn = sb.tile([P, FC], F32, tag="dn")
            lg = sb.tile([P, FC], F32, tag="lg")
            rc = sb.tile([P, FC], F32, tag="rc")

            # ---- loads ----
            for c in range(C):
                nc.sync.dma_start(xt[:, c], x_v[c, :, lo:lo + FC])
            nc.sync.dma_start(tt[:], t_v[:, lo:lo + FC])

            tt_g = tt[:].rearrange("p (g j) -> p g j", j=JB)

            def xg(c):
                return xt[:, c].rearrange("p (g j) -> p g j", j=JB)

            # ---- bf16 copies of raw logits (before in-place exp) ----
            for c in range(C):
                if c % 2 == 0:
                    nc.scalar.copy(gx[:, :, 12 + c], xg(c))
                else:
                    nc.vector.tensor_copy(gx[:, :, 12 + c], xg(c))

            # ---- masks m_t = (tgt == t) and the ones plane (DVE) ----
            for t in range(C):
                nc.vector.tensor_scalar(mk[:, :, t], tt_g, float(t), None,
                                        op0=ALU.is_equal)
            nc.vector.memset(mk[:, :, 12], 1.0)

            # ---- softmax pieces ----
            for c in range(C):
                nc.scalar.activation(xt[:, c], xt[:, c], ACTF.Exp)
            # denom over the class axis (strided innermost reduce)
            nc.vector.tensor_reduce(dn[:], xt[:].rearrange("p c f -> p f c"),
                                    axis=mybir.AxisListType.X, op=ALU.add)
            nc.scalar.activation(lg[:], dn[:], ACTF.Ln,
                                 accum_out=logdacc[:, ch:ch + 1])
            nc.scalar.activation(rc[:], lg[:], ACTF.Exp, scale=-1.0)

            # ---- probs g_c = exp_c * recip (bf16 out) ----
            rc_g = rc[:].rearrange("p (g j) -> p g j", j=JB)
            for c in range(C):
                nc.vector.tensor_tensor(gx[:, :, c], xg(c), rc_g,
                                        op=ALU.mult)

            # ---- accumulate the statistics matrix on PE, JB columns/op ----
            for g in range(FC // JB):
                nc.tensor.matmul(psum_m[:], mk[:, g], gx[:, g],
                                 start=(ch == 0 and g == 0),
                                 stop=(ch == NCH - 1 and g == FC // JB - 1))

        m_sb = acc.tile([13 * JB, 24 * JB], F32)
        nc.vector.tensor_copy(m_sb[:], psum_m[:])
        nc.sync.dma_start(m_d[:], m_sb[:])
        nc.sync.dma_start(logd_d[:], logdacc[:])

    nc.compile()
    return nc


def _get_program():
    if "nc" not in _CACHE:
        _CACHE["nc"] = _build_program()
    return _CACHE["nc"]


def _shard_inputs(input, target):
    """Full (2,12,96,96,96)/(2,1,96,96,96) -> 8 per-core in_maps."""
    x = np.ascontiguousarray(np.asarray(input, dtype=np.float32))
    tg = np.asarray(target).reshape(N, SPATIAL)
    in_maps = []
    for k in range(N_CORES):
        n = k // CORES_PER_N
        o = (k % CORES_PER_N) * S
        xs = np.ascontiguousarray(x[n].reshape(C, SPATIAL)[:, o:o + S])
        ts = np.ascontiguousarray(tg[n, o:o + S]).astype(np.float32)
        in_maps.append({"x": xs, "t": ts})
    return in_maps


def _combine(results, matrix_penalty, global_step, maxiter):
    pen = np.asarray(matrix_penalty, dtype=np.float64)
    inter = np.zeros((N, C))
    ground = np.zeros((N, C))
    pred = np.zeros((N, C))
    xtgt_sum = 0.0
    logd_sum = 0.0
    pen_sum = 0.0
    for k, r in enumerate(results):
        n = k // CORES_PER_N
        mfull = np.asarray(r["m_out"], dtype=np.float64) \
            .reshape(13, JB, 24, JB)
        m = np.einsum("sjcj->sc", mfull)        # sum the 9 diagonal blocks
        mg = m[:C, :C]                          # sum_v m_t * g_c
        inter[n] += np.diag(mg)
        ground[n] += mg.sum(axis=1)
        pred[n] += m[12, :C]
        xtgt_sum += np.trace(m[:C, C:2 * C])
        logd_sum += float(np.asarray(r["logd_out"], dtype=np.float64).sum())
        pen_sum += float((pen * mg).sum())

    nvox = N * SPATIAL
    dice = 1.0 - (2.0 * inter + 1e-5) / (ground + pred + 1e-5)
    dice_loss = dice.mean()
    ce = (logd_sum - xtgt_sum) / nvox
    ce_total = dice_loss + ce
    pen_mean = pen_sum / nvox
    beta = 10.0 ** np.floor(np.log10(ce_total))
    gs = float(global_step)
    mi = float(maxiter)
    alpha0 = 1.0 - gs / mi
    alpha1 = gs / mi
    return np.float32(alpha1 * ce_total + alpha0 * beta * pen_mean)


def kernel(input, target, matrix_penalty, global_step, maxiter):
    nc = _get_program()
    in_maps = _shard_inputs(input, target)
    trace = bool(int(os.environ.get("BASS_LOSS_TRACE", "0")))
    res = bass_utils.run_bass_kernel_spmd(
        nc, in_maps, core_ids=list(range(N_CORES)), trace=trace)
    _CACHE["last_exec_ns"] = res.exec_time_ns
    return _combine(res.results, matrix_penalty, global_step, maxiter)


# revision 8
# speedup vs baseline: 4.6131x; 1.3820x over previous
"""DOMINO++ loss kernel for Trainium2 (8 NeuronCores, data-parallel).

Strategy
--------
Shard the (n=2, c=12, 96^3) logits over 8 cores: 4 contiguous spatial
blocks per batch element (cores 0-3 -> n=0, cores 4-7 -> n=1).  Inputs
ship as bf16 (halves DMA; the per-element rounding noise statistically
cancels in the ~2e4..2e5-element sums this kernel reduces to).
Each core reduces its 221184 voxels to a small set of partial sums:

  - A [96, 192] PSUM accumulated on the TensorEngine holding 8 diagonal
    [12, 24] blocks (JB=8 voxel-columns batched per matmul; host sums
    the blocks):
      lhsT (stationary) = onehot masks m_t (12 planes x 8 columns)
      rhs  (moving)     = probs g_c | raw logits x_c
      M[t, c]    = sum_v m_t(v) g_c(v)   dice: inter = diag, ground =
                                         row sums, pred = col sums (masks
                                         are a partition of unity);
                                         penalty = <P, M> Frobenius
      X[t, t]    = sum_{v in t} x_t(v)   CE logit-gather term
  - logdacc[128, NCH] f32: row sums of log(sum_c exp x_c) (ACT accum_out)

Host combines the 8 cores' tiny outputs into the scalar loss.

Engine split per chunk (tiles [128, F] voxel-major, all bulk data bf16):
  DMA: 12 full-height class rows, contiguous 3456B runs, spread over the
       sync/scalar HWDGE + gpsimd SWDGE queues
  DVE: interleave-copy of x for PE; 12 onehot masks; 4 merged tree-adds
       for the softmax denominator; 12 prob muls
  ACT: 12 exp (in-place), ln(denom)+accum, recip = exp(-ln d)
  PE:  2 matmuls per 8-column group (probs block, x block)
"""

import os
import sys
from contextlib import ExitStack

import numpy as np

sys.path.insert(0, "/opt/trn_rl_repo")

from concourse import bacc, bass, mybir, tile  # noqa: E402
from concourse import bass_utils  # noqa: E402

F32 = mybir.dt.float32
BF16 = mybir.dt.bfloat16
ALU = mybir.AluOpType
ACTF = mybir.ActivationFunctionType

N_CORES = 8
C = 12            # classes
P = 128           # SBUF partitions
FT = 1728         # free size per partition per core (P*FT = 221184 voxels)
NCH = 4           # chunks
FC = FT // NCH    # free columns per chunk (432)
JB = 8            # voxel-columns batched per matmul (12*JB <= 128)
G = FC // JB      # matmul groups per chunk (54)
S = P * FT        # voxels per core
N, H, W, Z = 2, 96, 96, 96
SPATIAL = H * W * Z          # 884736 voxels per batch element
CORES_PER_N = N_CORES // N   # 4

_CACHE = {}


def _build_program():
    """Build + compile the per-core Bass program (identical on all cores)."""
    nc = bacc.Bacc("TRN2", target_bir_lowering=False, debug=False,
                   num_devices=N_CORES)

    x_d = nc.dram_tensor("x", (C, S), BF16, kind="ExternalInput")
    t_d = nc.dram_tensor("t", (S,), BF16, kind="ExternalInput")
    m_d = nc.dram_tensor("m_out", (12 * JB, 24 * JB), F32,
                         kind="ExternalOutput")
    logd_d = nc.dram_tensor("logd_out", (P, NCH), F32, kind="ExternalOutput")

    x_v = x_d.rearrange("c (p f) -> c p f", p=P)
    t_v = t_d.rearrange("(p f) -> p f", p=P)

    with ExitStack() as ctx:
        tc = ctx.enter_context(tile.TileContext(nc))
        sb = ctx.enter_context(tc.tile_pool(name="sb", bufs=2))
        acc = ctx.enter_context(tc.tile_pool(name="acc", bufs=1))
        ps = ctx.enter_context(tc.tile_pool(name="ps", bufs=1, space="PSUM"))

        logdacc = acc.tile([P, NCH], F32)
        ps_g = ps.tile([12 * JB, 12 * JB], F32)
        ps_x = ps.tile([12 * JB, 12 * JB], F32)

        # full-height input tiles: 13 big DMAs with 3456B contiguous runs,
        # spread across the three available DGE queues
        xt = acc.tile([P, C, FT], BF16)
        tt = acc.tile([P, FT], BF16)
        for c in range(C):
            eng = (nc.sync, nc.scalar, nc.gpsimd)[c % 3]
            eng.dma_start(xt[:, c], x_v[c])
        nc.sync.dma_start(tt[:], t_v[:])

        for ch in range(NCH):
            sl = slice(ch * FC, (ch + 1) * FC)
            xi = sb.tile([P, G, C, JB], BF16, tag="xi")
            gm = sb.tile([P, G, C, JB], BF16, tag="gm")
            mk = sb.tile([P, G, C, JB], BF16, tag="mk")
            tmp6 = sb.tile([P, 6, FC], BF16, tag="tmp6")
            tmp3 = sb.tile([P, 3, FC], BF16, tag="tmp3")
            dna = sb.tile([P, FC], BF16, tag="dna")
            dn = sb.tile([P, FC], BF16, tag="dn")
            lg = sb.tile([P, FC], F32, tag="lg")
            rc = sb.tile([P, FC], BF16, tag="rc")

            # interleaved copy of the raw logits for the PE x-block
            # (one op: (c, g, j) strides on both sides)
            nc.vector.tensor_copy(
                xi[:].rearrange("p g c j -> p c g j"),
                xt[:, :, sl].rearrange("p c (g j) -> p c g j", j=JB))

            # onehot masks
            tg = tt[:, sl].rearrange("p (g j) -> p g j", j=JB)
            for t in range(C):
                nc.vector.tensor_scalar(mk[:, :, t], tg, float(t), None,
                                        op0=ALU.is_equal)

            # exp in-place (after xi snapshot; Tile orders the WAR dep)
            for c in range(C):
                nc.scalar.activation(xt[:, c, sl], xt[:, c, sl], ACTF.Exp)

            # softmax denominator: merged pairwise tree adds
            nc.vector.tensor_tensor(tmp6[:], xt[:, 0::2, sl], xt[:, 1::2, sl],
                                    op=ALU.add)
            nc.vector.tensor_tensor(tmp3[:], tmp6[:, 0::2], tmp6[:, 1::2],
                                    op=ALU.add)
            nc.vector.tensor_tensor(dna[:], tmp3[:, 0], tmp3[:, 1],
                                    op=ALU.add)
            nc.vector.tensor_tensor(dn[:], dna[:], tmp3[:, 2], op=ALU.add)

            nc.scalar.activation(lg[:], dn[:], ACTF.Ln,
                                 accum_out=logdacc[:, ch:ch + 1])
            nc.scalar.activation(rc[:], lg[:], ACTF.Exp, scale=-1.0)

            # probs g_c = exp_c * recip
            rc_g = rc[:].rearrange("p (g j) -> p g j", j=JB)
            for c in range(C):
                nc.vector.tensor_tensor(gm[:, :, c],
                                        xt[:, c, sl].rearrange(
                                            "p (g j) -> p g j", j=JB),
                                        rc_g, op=ALU.mult)

            # statistics matrix on PE: probs block + x block
            for g in range(G):
                first = ch == 0 and g == 0
                last = ch == NCH - 1 and g == G - 1
                nc.tensor.matmul(ps_g[:], mk[:, g], gm[:, g],
                                 start=first, stop=last)
                nc.tensor.matmul(ps_x[:], mk[:, g], xi[:, g],
                                 start=first, stop=last)

        m_sb = acc.tile([12 * JB, 24 * JB], F32)
        nc.vector.tensor_copy(m_sb[:, 0:12 * JB], ps_g[:])
        nc.vector.tensor_copy(m_sb[:, 12 * JB:], ps_x[:])
        nc.sync.dma_start(m_d[:], m_sb[:])
        nc.sync.dma_start(logd_d[:], logdacc[:])

    nc.compile()
    return nc


def _get_program():
    if "nc" not in _CACHE:
        _CACHE["nc"] = _build_program()
    return _CACHE["nc"]


def _shard_inputs(input, target):
    """Full (2,12,96,96,96)/(2,1,96,96,96) -> 8 per-core in_maps (bf16)."""
    bf16 = mybir.dt.np(BF16)
    x = np.asarray(input, dtype=np.float32)
    tg = np.asarray(target).reshape(N, SPATIAL)
    in_maps = []
    for k in range(N_CORES):
        n = k // CORES_PER_N
        o = (k % CORES_PER_N) * S
        xs = np.ascontiguousarray(
            x[n].reshape(C, SPATIAL)[:, o:o + S]).astype(bf16)
        ts = np.ascontiguousarray(tg[n, o:o + S]).astype(np.float32) \
            .astype(bf16)
        in_maps.append({"x": xs, "t": ts})
    return in_maps


def _combine(results, matrix_penalty, global_step, maxiter):
    pen = np.asarray(matrix_penalty, dtype=np.float64)
    inter = np.zeros((N, C))
    ground = np.zeros((N, C))
    pred = np.zeros((N, C))
    xtgt_sum = 0.0
    logd_sum = 0.0
    pen_sum = 0.0
    for k, r in enumerate(results):
        n = k // CORES_PER_N
        mfull = np.asarray(r["m_out"], dtype=np.float64) \
            .reshape(C, JB, 2 * C, JB)
        m = np.einsum("tjcj->tc", mfull)        # sum the JB diagonal blocks
        mg = m[:, :C]                           # sum_v m_t * g_c
        inter[n] += np.diag(mg)
        ground[n] += mg.sum(axis=1)
        pred[n] += mg.sum(axis=0)               # masks partition unity
        xtgt_sum += np.trace(m[:, C:2 * C])
        logd_sum += float(np.asarray(r["logd_out"], dtype=np.float64).sum())
        pen_sum += float((pen * mg).sum())

    nvox = N * SPATIAL
    dice = 1.0 - (2.0 * inter + 1e-5) / (ground + pred + 1e-5)
    dice_loss = dice.mean()
    ce = (logd_sum - xtgt_sum) / nvox
    ce_total = dice_loss + ce
    pen_mean = pen_sum / nvox
    beta = 10.0 ** np.floor(np.log10(ce_total))
    gs = float(global_step)
    mi = float(maxiter)
    alpha0 = 1.0 - gs / mi
    alpha1 = gs / mi
    return np.float32(alpha1 * ce_total + alpha0 * beta * pen_mean)


def kernel(input, target, matrix_penalty, global_step, maxiter):
    nc = _get_program()
    in_maps = _shard_inputs(input, target)
    trace = bool(int(os.environ.get("BASS_LOSS_TRACE", "0")))
    res = bass_utils.run_bass_kernel_spmd(
        nc, in_maps, core_ids=list(range(N_CORES)), trace=trace)
    _CACHE["last_exec_ns"] = res.exec_time_ns
    return _combine(res.results, matrix_penalty, global_step, maxiter)
